# revision 4
# baseline (speedup 1.0000x reference)
"""Trainium2 Bass kernel for nn_DiscriminativeLoss (segment_reduce).

Strategy (stage 1): 8 (batch, task) units -> 8 cores. Each core computes,
for its [D=5, N=524288] prediction plane + labels:
  - counts[k], sums[k,d]  (segment sums, fused mask-dot ops on DVE)
  - on-core means -> per-pixel hinge distances -> V[k] partials
Host does only the tiny O(K*D) finishing math (means, pairwise term, reg).

Binary task units are padded to K=6/D=5 with zero planes so one SPMD
program serves all cores; host slices the first K=2 rows of the partials.
"""

import numpy as np
import ml_dtypes

import concourse.bacc as bacc
import concourse.bass as bass
import concourse.bass_isa as bass_isa
import concourse.mybir as mybir
import concourse.tile as tile
from concourse import bass_utils

F32 = mybir.dt.float32
BF16 = mybir.dt.bfloat16
NP_BF16 = ml_dtypes.bfloat16

P = 128          # SBUF partitions
N = 512 * 1024   # pixels per (batch, task)
F = N // P       # free dim per partition (4096)
K = 6            # padded label count
D = 5            # padded feature dim
B = 4

DELTA_V = 0.5
DELTA_D = 3.0
PARAM_VAR = 1.0
PARAM_DIST = 1.0
PARAM_REG = 0.001

# per-partition partial layout (per core output, f32, summed on host):
#   [0:6)    counts
#   [6:36)   sums (k*5 + d)
#   [36:42)  V_k (hinge segment sums)
NPART = 42

_compiled = {}


def _build_module():
    nc = bacc.Bacc("TRN2", target_bir_lowering=False, debug=False, num_devices=8)

    x_dram = nc.dram_tensor("x", [P, D, F], BF16, kind="ExternalInput")
    lab_dram = nc.dram_tensor("lab", [P, F], BF16, kind="ExternalInput")
    out_dram = nc.dram_tensor("out", [P, NPART], F32, kind="ExternalOutput")

    AL = mybir.AluOpType
    AF = mybir.ActivationFunctionType

    with tile.TileContext(nc) as tc:
        with (
            tc.tile_pool(name="data", bufs=1) as data_pool,
            tc.tile_pool(name="scr", bufs=2) as scr_pool,
            tc.tile_pool(name="zp", bufs=2) as z_pool,
            tc.tile_pool(name="dp", bufs=3) as d_pool,
            tc.tile_pool(name="small", bufs=1) as small_pool,
        ):
            xt = data_pool.tile([P, D, F], BF16, tag="xt")
            lab = data_pool.tile([P, F], BF16, tag="lab")
            acc = small_pool.tile([P, NPART], F32, tag="acc")

            nc.sync.dma_start(xt[:], x_dram[:])
            nc.sync.dma_start(lab[:], lab_dram[:])

            # ---------------- phase 1: counts + segment sums -------------
            for k in range(K):
                scratch = scr_pool.tile([P, F], BF16, tag="work")
                nc.vector.tensor_scalar(
                    out=scratch[:],
                    in0=lab[:],
                    scalar1=float(k),
                    scalar2=None,
                    op0=AL.is_equal,
                    op1=AL.add,
                    accum_out=acc[:, k : k + 1],
                )
            for k in range(K):
                for d in range(D):
                    scratch = scr_pool.tile([P, F], BF16, tag="work")
                    nc.vector.scalar_tensor_tensor(
                        out=scratch[:],
                        in0=lab[:],
                        scalar=float(k),
                        in1=xt[:, d, :],
                        op0=AL.is_equal,
                        op1=AL.mult,
                        accum_out=acc[:, 6 + k * D + d : 7 + k * D + d],
                    )

            # all-reduce phase-1 partials across partitions (tiny)
            stats = small_pool.tile([P, 36], F32, tag="stats")
            nc.gpsimd.partition_all_reduce(
                stats[:], acc[:, 0:36], channels=P, reduce_op=bass_isa.ReduceOp.add
            )

            # stats[:, 0:6] counts ; stats[:, 6:36] sums   (same on every partition)
            csafe = small_pool.tile([P, K], F32, tag="csafe")
            crecip = small_pool.tile([P, K], F32, tag="crecip")
            nc.vector.tensor_scalar(
                out=csafe[:], in0=stats[:, 0:6], scalar1=0.5, scalar2=None, op0=AL.max
            )
            nc.vector.reciprocal(crecip[:], csafe[:])

            mu = small_pool.tile([P, K, D], F32, tag="mu")      # means
            m2mu = small_pool.tile([P, K, D], F32, tag="m2mu")  # -2*means
            musq = small_pool.tile([P, K, D], F32, tag="musq")
            a_k = small_pool.tile([P, K], F32, tag="a_k")       # ||mu_k||^2
            sums_v = stats[:, 6:36].rearrange("p (k d) -> p k d", k=K)
            for d in range(D):
                nc.vector.tensor_tensor(
                    out=mu[:, :, d], in0=sums_v[:, :, d], in1=crecip[:], op=AL.mult
                )
            nc.vector.tensor_scalar(
                out=m2mu[:], in0=mu[:], scalar1=-2.0, scalar2=None, op0=AL.mult
            )
            nc.vector.tensor_tensor(out=musq[:], in0=mu[:], in1=mu[:], op=AL.mult)
            nc.vector.tensor_reduce(
                out=a_k[:], in_=musq[:], axis=mybir.AxisListType.X, op=AL.add
            )

            # ---------------- phase 2: per-pixel hinge ------------------
            # dacc = sum_d x_d^2   (squares on ACT, adds on DVE)
            dacc = None
            prev_sq = None
            for d in range(D):
                sq = scr_pool.tile([P, F], BF16, tag="work")
                nc.scalar.square(sq[:], xt[:, d, :])
                if prev_sq is None:
                    prev_sq = sq
                    continue
                nxt = d_pool.tile([P, F], F32, tag="dacc")
                if dacc is None:
                    nc.vector.tensor_tensor(
                        out=nxt[:], in0=prev_sq[:], in1=sq[:], op=AL.add
                    )
                else:
                    nc.vector.tensor_tensor(out=nxt[:], in0=dacc[:], in1=sq[:], op=AL.add)
                dacc = nxt
                prev_sq = sq

            # dacc += sum_k m_k * (A_k - 2 mu_k . x)
            for k in range(K):
                z = z_pool.tile([P, F], BF16, tag="zwork")
                nc.vector.tensor_scalar(
                    out=z[:],
                    in0=xt[:, 0, :],
                    scalar1=m2mu[:, k, 0:1],
                    scalar2=a_k[:, k : k + 1],
                    op0=AL.mult,
                    op1=AL.add,
                )
                for d in range(1, D):
                    z2 = z_pool.tile([P, F], BF16, tag="zwork")
                    nc.vector.scalar_tensor_tensor(
                        out=z2[:],
                        in0=xt[:, d, :],
                        scalar=m2mu[:, k, d : d + 1],
                        in1=z[:],
                        op0=AL.mult,
                        op1=AL.add,
                    )
                    z = z2
                zm = z_pool.tile([P, F], BF16, tag="zwork")
                nc.vector.scalar_tensor_tensor(
                    out=zm[:],
                    in0=lab[:],
                    scalar=float(k),
                    in1=z[:],
                    op0=AL.is_equal,
                    op1=AL.mult,
                )
                nxt = d_pool.tile([P, F], F32, tag="dacc")
                nc.vector.tensor_tensor(out=nxt[:], in0=dacc[:], in1=zm[:], op=AL.add)
                dacc = nxt

            # hinge: h = relu(sqrt(max(dist2,0)) - delta_v)^2
            dcl = d_pool.tile([P, F], F32, tag="dacc")
            nc.vector.tensor_scalar(
                out=dcl[:], in0=dacc[:], scalar1=0.0, scalar2=None, op0=AL.max
            )
            dist = d_pool.tile([P, F], F32, tag="dacc")
            nc.scalar.activation(dist[:], dcl[:], AF.Sqrt)
            negdv = small_pool.tile([P, 1], F32, tag="negdv")
            nc.gpsimd.memset(negdv[:], -DELTA_V)
            hr = d_pool.tile([P, F], F32, tag="dacc")
            nc.scalar.activation(hr[:], dist[:], AF.Relu, bias=negdv[:])
            h = d_pool.tile([P, F], F32, tag="dacc")
            nc.scalar.square(h[:], hr[:])

            # V_k partials
            for k in range(K):
                vout = z_pool.tile([P, F], BF16, tag="zwork")
                nc.vector.scalar_tensor_tensor(
                    out=vout[:],
                    in0=lab[:],
                    scalar=float(k),
                    in1=h[:],
                    op0=AL.is_equal,
                    op1=AL.mult,
                    accum_out=acc[:, 36 + k : 37 + k],
                )

            nc.sync.dma_start(out_dram[:], acc[:])

    nc.compile()
    return nc


def _get_module():
    if "nc" not in _compiled:
        _compiled["nc"] = _build_module()
    return _compiled["nc"]


def _prep_core_inputs(x, labels):
    """x: [d, N] f32 (d<=D), labels: [N] int -> in_map dict."""
    d = x.shape[0]
    xp = np.zeros((D, P, F), dtype=NP_BF16)
    xp[:d] = np.asarray(x, dtype=np.float32).reshape(d, P, F).astype(NP_BF16)
    xp = np.ascontiguousarray(xp.transpose(1, 0, 2))  # [P, D, F]
    lb = np.ascontiguousarray(
        np.asarray(labels).astype(np.float32).reshape(P, F).astype(NP_BF16)
    )
    return {"x": xp, "lab": lb}


def _finish_loss(partials, k_eff):
    """partials: [NPART] f64 (partition-summed), k_eff: 2 or 6 -> loss."""
    p = partials.astype(np.float64)
    counts = p[0:k_eff]
    sums = p[6:36].reshape(K, D)[:k_eff]
    V = p[36 : 36 + k_eff]
    means = sums / counts[:, None]

    l_var = np.mean(V / counts)

    diff = means[:, None, :] - means[None, :, :]
    sq = np.sum(diff**2, axis=-1)
    offdiag = ~np.eye(k_eff, dtype=bool)
    dn = np.maximum(2.0 * DELTA_D - np.sqrt(np.where(offdiag, sq, 1.0)), 0.0) ** 2
    l_dist = np.sum(np.where(offdiag, dn, 0.0)) / (k_eff * (k_eff - 1))

    l_reg = np.mean(np.sqrt(np.sum(means**2, axis=1)))

    return PARAM_VAR * l_var + PARAM_DIST * l_dist + PARAM_REG * l_reg


def kernel(binary_logits, binary_labels, instance_logits, instance_labels):
    binary_logits = np.asarray(binary_logits, dtype=np.float32)
    instance_logits = np.asarray(instance_logits, dtype=np.float32)
    binary_labels = np.asarray(binary_labels)
    instance_labels = np.asarray(instance_labels)

    nc = _get_module()

    in_maps = []
    for b in range(B):  # cores 0-3: instance units
        in_maps.append(_prep_core_inputs(instance_logits[b], instance_labels[b]))
    for b in range(B):  # cores 4-7: binary units
        in_maps.append(_prep_core_inputs(binary_logits[b], binary_labels[b]))

    res = bass_utils.run_bass_kernel_spmd(nc, in_maps, core_ids=list(range(8)))

    inst_losses = []
    bin_losses = []
    for c in range(B):
        inst_losses.append(_finish_loss(res.results[c]["out"].sum(axis=0), 6))
    for c in range(B):
        bin_losses.append(_finish_loss(res.results[B + c]["out"].sum(axis=0), 2))

    return (
        np.float32(np.mean(bin_losses)),
        np.float32(np.mean(inst_losses)),
    )


# revision 5
# speedup vs baseline: 108.5976x; 108.5976x over previous
"""Trainium2 Bass kernel for nn_DiscriminativeLoss (segment_reduce).

Strategy: 8 (batch, task) units -> 8 cores. Each core computes, for its
[D=5, N=524288] prediction plane + labels:
  - counts[k], sums[k,d]  (fused mask-dot ops on DVE; k=0 recovered from
    ACT-accumulated totals)
  - on-core means -> per-pixel hinge distances -> V[k] partials
Host does only the tiny O(K*D) finishing math (means, pairwise term, reg).

Binary task units are padded to K=6/D=5 with zero planes so one SPMD
program serves all cores; host slices the first K=2 rows of the partials.
"""

import numpy as np
import ml_dtypes

import concourse.bacc as bacc
import concourse.bass as bass
import concourse.bass_isa as bass_isa
import concourse.mybir as mybir
import concourse.tile as tile
from concourse import bass_utils

F32 = mybir.dt.float32
BF16 = mybir.dt.bfloat16
NP_BF16 = ml_dtypes.bfloat16

P = 128          # SBUF partitions
N = 512 * 1024   # pixels per (batch, task)
F = N // P       # free dim per partition (4096)
K = 6            # padded label count
D = 5            # padded feature dim
B = 4

DELTA_V = 0.5
DELTA_D = 3.0
PARAM_VAR = 1.0
PARAM_DIST = 1.0
PARAM_REG = 0.001

# per-partition partial layout (per core output, f32, summed on host):
#   [0:5)    counts k=1..5
#   [5:30)   sums (k-1)*5+d, k=1..5
#   [30:35)  totals_d = sum_n x_d
#   [35:40)  V_k k=1..5
#   [40]     V_tot = sum_n h
NPART = 41
C_CNT, C_SUM, C_TOT, C_V, C_VT = 0, 5, 30, 35, 40

_compiled = {}


def _build_module():
    nc = bacc.Bacc("TRN2", target_bir_lowering=False, debug=False, num_devices=8)

    x_dram = nc.dram_tensor("x", [P, D, F], BF16, kind="ExternalInput")
    lab_dram = nc.dram_tensor("lab", [P, F], BF16, kind="ExternalInput")
    out_dram = nc.dram_tensor("out", [P, NPART], F32, kind="ExternalOutput")

    AL = mybir.AluOpType
    AF = mybir.ActivationFunctionType

    with tile.TileContext(nc) as tc:
        with (
            tc.tile_pool(name="data", bufs=1) as data_pool,
            tc.tile_pool(name="scr", bufs=2) as scr_pool,
            tc.tile_pool(name="zp", bufs=2) as z_pool,
            tc.tile_pool(name="dp", bufs=3) as d_pool,
            tc.tile_pool(name="small", bufs=1) as small_pool,
        ):
            xt = data_pool.tile([P, D, F], BF16, tag="xt")
            lab = data_pool.tile([P, F], BF16, tag="lab")
            acc = small_pool.tile([P, NPART], F32, tag="acc")

            nc.sync.dma_start(lab[:], lab_dram[:])
            for d in range(D):
                nc.sync.dma_start(xt[:, d, :], x_dram[:, d, :])

            # ---------------- phase 1: counts + segment sums -------------
            for k in range(1, K):
                scratch = scr_pool.tile([P, F], BF16, tag="work")
                nc.vector.tensor_scalar(
                    out=scratch[:],
                    in0=lab[:],
                    scalar1=float(k),
                    scalar2=None,
                    op0=AL.is_equal,
                    op1=AL.add,
                    accum_out=acc[:, C_CNT + k - 1 : C_CNT + k],
                )
            for d in range(D):
                # totals on ACT (parallel engine)
                tsc = scr_pool.tile([P, F], BF16, tag="twork")
                nc.scalar.activation(
                    tsc[:], xt[:, d, :], AF.Copy,
                    accum_out=acc[:, C_TOT + d : C_TOT + d + 1],
                )
                for k in range(1, K):
                    scratch = scr_pool.tile([P, F], BF16, tag="work")
                    nc.vector.scalar_tensor_tensor(
                        out=scratch[:],
                        in0=lab[:],
                        scalar=float(k),
                        in1=xt[:, d, :],
                        op0=AL.is_equal,
                        op1=AL.mult,
                        accum_out=acc[
                            :, C_SUM + (k - 1) * D + d : C_SUM + (k - 1) * D + d + 1
                        ],
                    )

            # all-reduce phase-1 partials across partitions (tiny)
            stats = small_pool.tile([P, 35], F32, tag="stats")
            nc.gpsimd.partition_all_reduce(
                stats[:], acc[:, 0:35], channels=P, reduce_op=bass_isa.ReduceOp.add
            )

            # rebuild k=0 row: count_0 = N - sum(counts), sums_0 = tot - sum_k
            cnt_full = small_pool.tile([P, K], F32, tag="cnt_full")
            sums_full = small_pool.tile([P, K, D], F32, tag="sums_full")
            red = small_pool.tile([P, 1], F32, tag="red")
            nc.vector.tensor_reduce(
                out=red[:], in_=stats[:, C_CNT : C_CNT + 5],
                axis=mybir.AxisListType.X, op=AL.add,
            )
            nc.vector.tensor_scalar(
                out=cnt_full[:, 0:1], in0=red[:], scalar1=-1.0, scalar2=float(N),
                op0=AL.mult, op1=AL.add,
            )
            nc.vector.tensor_copy(cnt_full[:, 1:6], stats[:, C_CNT : C_CNT + 5])
            sums_kd = stats[:, C_SUM : C_SUM + 25].rearrange(
                "p (k d) -> p d k", k=5
            )  # [P, d, k] strided view
            redd = small_pool.tile([P, D], F32, tag="redd")
            nc.vector.tensor_reduce(
                out=redd[:], in_=sums_kd, axis=mybir.AxisListType.X, op=AL.add
            )
            nc.vector.tensor_sub(
                sums_full[:, 0, :], stats[:, C_TOT : C_TOT + 5], redd[:]
            )
            nc.vector.tensor_copy(
                sums_full[:, 1:6, :],
                stats[:, C_SUM : C_SUM + 25].rearrange("p (k d) -> p k d", k=5),
            )

            csafe = small_pool.tile([P, K], F32, tag="csafe")
            crecip = small_pool.tile([P, K], F32, tag="crecip")
            nc.vector.tensor_scalar(
                out=csafe[:], in0=cnt_full[:], scalar1=0.5, scalar2=None, op0=AL.max
            )
            nc.vector.reciprocal(crecip[:], csafe[:])

            mu = small_pool.tile([P, K, D], F32, tag="mu")      # means
            m2mu = small_pool.tile([P, K, D], F32, tag="m2mu")  # -2*means
            musq = small_pool.tile([P, K, D], F32, tag="musq")
            a_k = small_pool.tile([P, K], F32, tag="a_k")       # ||mu_k||^2
            for d in range(D):
                nc.vector.tensor_tensor(
                    out=mu[:, :, d], in0=sums_full[:, :, d], in1=crecip[:], op=AL.mult
                )
            nc.vector.tensor_scalar(
                out=m2mu[:], in0=mu[:], scalar1=-2.0, scalar2=None, op0=AL.mult
            )
            nc.vector.tensor_tensor(out=musq[:], in0=mu[:], in1=mu[:], op=AL.mult)
            nc.vector.tensor_reduce(
                out=a_k[:], in_=musq[:], axis=mybir.AxisListType.X, op=AL.add
            )

            # ---------------- phase 2: per-pixel hinge ------------------
            # dacc = sum_d x_d^2   (squares on ACT, adds on DVE)
            dacc = None
            prev_sq = None
            for d in range(D):
                sq = scr_pool.tile([P, F], BF16, tag="twork")
                nc.scalar.square(sq[:], xt[:, d, :])
                if prev_sq is None:
                    prev_sq = sq
                    continue
                nxt = d_pool.tile([P, F], BF16, tag="dacc")
                if dacc is None:
                    nc.vector.tensor_tensor(
                        out=nxt[:], in0=prev_sq[:], in1=sq[:], op=AL.add
                    )
                else:
                    nc.vector.tensor_tensor(out=nxt[:], in0=dacc[:], in1=sq[:], op=AL.add)
                dacc = nxt
                prev_sq = sq

            # dacc += sum_k m_k * (A_k - 2 mu_k . x)
            for k in range(K):
                z = z_pool.tile([P, F], BF16, tag="zwork")
                nc.vector.tensor_scalar(
                    out=z[:],
                    in0=xt[:, 0, :],
                    scalar1=m2mu[:, k, 0:1],
                    scalar2=a_k[:, k : k + 1],
                    op0=AL.mult,
                    op1=AL.add,
                )
                for d in range(1, D):
                    z2 = z_pool.tile([P, F], BF16, tag="zwork")
                    nc.vector.scalar_tensor_tensor(
                        out=z2[:],
                        in0=xt[:, d, :],
                        scalar=m2mu[:, k, d : d + 1],
                        in1=z[:],
                        op0=AL.mult,
                        op1=AL.add,
                    )
                    z = z2
                zm = z_pool.tile([P, F], BF16, tag="zwork")
                nc.vector.scalar_tensor_tensor(
                    out=zm[:],
                    in0=lab[:],
                    scalar=float(k),
                    in1=z[:],
                    op0=AL.is_equal,
                    op1=AL.mult,
                )
                nxt = d_pool.tile([P, F], BF16, tag="dacc")
                nc.vector.tensor_tensor(out=nxt[:], in0=dacc[:], in1=zm[:], op=AL.add)
                dacc = nxt

            # hinge: h = relu(sqrt(max(dist2,0)) - delta_v)^2
            dcl = d_pool.tile([P, F], BF16, tag="dacc")
            nc.vector.tensor_scalar(
                out=dcl[:], in0=dacc[:], scalar1=0.0, scalar2=None, op0=AL.max
            )
            dist = d_pool.tile([P, F], BF16, tag="dacc")
            nc.scalar.activation(dist[:], dcl[:], AF.Sqrt)
            negdv = small_pool.tile([P, 1], F32, tag="negdv")
            nc.gpsimd.memset(negdv[:], -DELTA_V)
            hr = d_pool.tile([P, F], BF16, tag="dacc")
            nc.scalar.activation(hr[:], dist[:], AF.Relu, bias=negdv[:])
            h = d_pool.tile([P, F], BF16, tag="dacc")
            nc.scalar.activation(
                h[:], hr[:], AF.Square, accum_out=acc[:, C_VT : C_VT + 1]
            )

            # V_k partials (k=1..5; V_0 = V_tot - sum on host)
            for k in range(1, K):
                vout = z_pool.tile([P, F], BF16, tag="zwork")
                nc.vector.scalar_tensor_tensor(
                    out=vout[:],
                    in0=lab[:],
                    scalar=float(k),
                    in1=h[:],
                    op0=AL.is_equal,
                    op1=AL.mult,
                    accum_out=acc[:, C_V + k - 1 : C_V + k],
                )

            nc.sync.dma_start(out_dram[:], acc[:])

    nc.compile()
    return nc


def _get_module():
    if "nc" not in _compiled:
        _compiled["nc"] = _build_module()
    return _compiled["nc"]


def _prep_core_inputs(x, labels):
    """x: [d, N] f32 (d<=D), labels: [N] int -> in_map dict."""
    d = x.shape[0]
    xp = np.zeros((D, P, F), dtype=NP_BF16)
    xp[:d] = np.asarray(x, dtype=np.float32).reshape(d, P, F).astype(NP_BF16)
    xp = np.ascontiguousarray(xp.transpose(1, 0, 2))  # [P, D, F]
    lb = np.ascontiguousarray(
        np.asarray(labels).astype(np.float32).reshape(P, F).astype(NP_BF16)
    )
    return {"x": xp, "lab": lb}


def _finish_loss(partials, k_eff):
    """partials: [NPART] (partition-summed), k_eff: 2 or 6 -> loss."""
    p = partials.astype(np.float64)
    counts5 = p[C_CNT : C_CNT + 5]
    sums5 = p[C_SUM : C_SUM + 25].reshape(5, D)
    tot = p[C_TOT : C_TOT + 5]
    V5 = p[C_V : C_V + 5]
    vtot = p[C_VT]

    counts = np.concatenate([[N - counts5.sum()], counts5])[:k_eff]
    sums = np.concatenate([(tot - sums5.sum(axis=0))[None], sums5])[:k_eff]
    V = np.concatenate([[vtot - V5.sum()], V5])[:k_eff]
    means = sums / counts[:, None]

    l_var = np.mean(V / counts)

    diff = means[:, None, :] - means[None, :, :]
    sq = np.sum(diff**2, axis=-1)
    offdiag = ~np.eye(k_eff, dtype=bool)
    dn = np.maximum(2.0 * DELTA_D - np.sqrt(np.where(offdiag, sq, 1.0)), 0.0) ** 2
    l_dist = np.sum(np.where(offdiag, dn, 0.0)) / (k_eff * (k_eff - 1))

    l_reg = np.mean(np.sqrt(np.sum(means**2, axis=1)))

    return PARAM_VAR * l_var + PARAM_DIST * l_dist + PARAM_REG * l_reg


def kernel(binary_logits, binary_labels, instance_logits, instance_labels):
    binary_logits = np.asarray(binary_logits, dtype=np.float32)
    instance_logits = np.asarray(instance_logits, dtype=np.float32)
    binary_labels = np.asarray(binary_labels)
    instance_labels = np.asarray(instance_labels)

    nc = _get_module()

    in_maps = []
    for b in range(B):  # cores 0-3: instance units
        in_maps.append(_prep_core_inputs(instance_logits[b], instance_labels[b]))
    for b in range(B):  # cores 4-7: binary units
        in_maps.append(_prep_core_inputs(binary_logits[b], binary_labels[b]))

    res = bass_utils.run_bass_kernel_spmd(nc, in_maps, core_ids=list(range(8)))

    inst_losses = []
    bin_losses = []
    for c in range(B):
        inst_losses.append(_finish_loss(res.results[c]["out"].sum(axis=0), 6))
    for c in range(B):
        bin_losses.append(_finish_loss(res.results[B + c]["out"].sum(axis=0), 2))

    return (
        np.float32(np.mean(bin_losses)),
        np.float32(np.mean(inst_losses)),
    )
